# revision 1
# baseline (speedup 1.0000x reference)
"""DistMult edge scoring on Trainium2 (8 NeuronCores).

score_e = src_emb[e]^T @ W[rel_e] @ dst_emb[e]   for E=100k edges.

Strategy
--------
Host (index-space preprocessing only — no embedding data is gathered on
host):
  - Sort edges by relation, shard the sorted list contiguously across the
    8 cores (data-parallel over edges).
  - Per core, bucket edges into 16 segments by (src_bank, dst_bank) where
    a bank is a num_nodes/4-row range of the node table (node ids must fit
    the int16 index of the HW gather instruction). Within a segment, edges
    stay relation-sorted and each relation run is padded to a multiple of
    128 so every 128-edge tile is single-relation and single-bank on both
    endpoints. Segment capacities are maxed across cores so all cores run
    one SPMD program.
  - The small relation matrices (64x64x64 = 1MB) are expanded per tile on
    host and streamed over the HWDGE path, keeping the SWDGE gather
    queues free for the 25.6MB node table.

Device (per core, SPMD):
  - dma_gather (int16, 4 SWDGE queues, <=1024 rows/instruction) of
    src/dst embedding rows, per bank segment.
  - per tile: PE transpose src -> srcT [d, e] (base partition 0);
    PE matmul U[e,k] = sum_d srcT[d,e] * W[r,d,k];
    per 8 tiles: DVE mul by dst and reduce over k -> scores.
Host: drop pad slots, unsort scores to the original edge order.
"""

import numpy as np

import concourse.bacc as bacc
import concourse.mybir as mybir
from concourse.bass_utils import run_bass_kernel_spmd
from concourse.masks import make_identity
from concourse.tile import TileContext

NCORES = 8
P = 128          # SBUF partitions / edges per tile
DIM = 64         # embedding dim
NUM_RELS = 64
NBANKS = 4
TILE_GROUP = 8   # tiles per compute super-tile (one PSUM bank: 8*64 f32)
CHUNK = 1024     # max indices per dma_gather instruction (SWDGE ring limit)
NQ = 4           # SWDGE queues

TRACE = False
LAST_RESULT = None

_BUILD_CACHE = {}


def _prepare(triplets, num_nodes):
    """Index-space prep. Returns per-core int16 gather streams + unsort map.

    Slot s (= tile j * 128 + p) of core c holds the edge at padded position
    s; tiles are relation- and bank-pure by construction.
    """
    t = np.asarray(triplets)
    E = t.shape[0]
    src = t[:, 0].astype(np.int64)
    rel = t[:, 1].astype(np.int64)
    dst = t[:, 2].astype(np.int64)
    bank = -(-num_nodes // NBANKS)  # equal banks, < 32768 for int16
    assert bank <= 32767

    order = np.argsort(rel, kind="stable")
    bounds = [round(c * E / NCORES) for c in range(NCORES + 1)]

    # segments by src bank only; dst goes through a two-stage gather
    core_data = []
    for c in range(NCORES):
        eidx = order[bounds[c]:bounds[c + 1]]
        seg = src[eidx] // bank
        segs = []
        for s in range(NBANKS):
            sel = eidx[seg == s]           # still rel-sorted (stable mask)
            r = rel[sel]
            n = len(sel)
            if n:
                change = np.flatnonzero(np.diff(r)) + 1
                starts = np.concatenate([[0], change])
                ends = np.concatenate([change, [n]])
                lens = ends - starts
                padlens = ((lens + P - 1) // P) * P
                offs = np.concatenate([[0], np.cumsum(padlens)])
                total = int(offs[-1])
                se = np.full(total, -1, np.int64)
                pos = (np.arange(n) - np.repeat(starts, lens)
                       + np.repeat(offs[:-1], lens))
                se[pos] = sel
                sr = np.repeat(r[starts], padlens)
            else:
                se = np.zeros(0, np.int64)
                sr = np.zeros(0, np.int64)
            segs.append((se, sr))
        core_data.append(segs)

    caps = []
    for s in range(NBANKS):
        cap = max(len(core_data[c][s][0]) for c in range(NCORES)) // P
        caps.append(int(cap))
    K = sum(caps)
    pad_tiles = (-K) % TILE_GROUP  # compute loop works in groups of 8 tiles
    caps[-1] += pad_tiles
    K += pad_tiles
    seg_tile0 = np.concatenate([[0], np.cumsum(caps)]).astype(int)

    nslot = K * P
    src_loc = np.zeros((NCORES, nslot), np.int64)   # bank-local src idx
    dst_all = np.zeros((NCORES, nslot), np.int64)   # global dst ids
    dst_valid = np.zeros((NCORES, nslot), bool)
    relc_all = np.zeros((NCORES, K), np.int64)      # relation per tile
    slot_edge_full = np.full((NCORES, nslot), -1, np.int64)

    for c in range(NCORES):
        for s in range(NBANKS):
            se, sr = core_data[c][s]
            a = seg_tile0[s] * P
            m = len(se)
            slot_edge_full[c, a:a + m] = se
            valid = se >= 0
            sl = np.zeros(m, np.int64)
            sl[valid] = src[se[valid]] - s * bank
            src_loc[c, a:a + m] = sl
            dl = np.zeros(m, np.int64)
            dl[valid] = dst[se[valid]]
            dst_all[c, a:a + m] = dl
            dst_valid[c, a:a + m] = valid
            nt = m // P
            if nt:
                relc_all[c, seg_tile0[s]:seg_tile0[s] + nt] = \
                    sr.reshape(nt, P)[:, 0]

    # dst stage A: per dst bank, the slots needing that bank, compacted.
    # Per-bank capacity maxed over cores; total rows CT (scratch size).
    bank_cnt = np.zeros((NCORES, NBANKS), np.int64)
    for c in range(NCORES):
        for b in range(NBANKS):
            sel = dst_valid[c] & (dst_all[c] // bank == b)
            bank_cnt[c, b] = len(np.unique(dst_all[c, sel]))
    bcaps = [int(-(-bank_cnt[:, b].max() // P) * P) for b in range(NBANKS)]
    brow0 = np.concatenate([[0], np.cumsum(bcaps)]).astype(int)
    CT = int(brow0[-1])  # scratch rows; must fit int16 for stage B
    assert CT <= 32767 and CT % P == 0

    dstA_loc = np.zeros((NCORES, CT), np.int64)   # stage-A bank-local ids
    dstB_row = np.zeros((NCORES, nslot), np.int64)  # stage-B scratch rows
    BC = CT // P  # scratch free blocks per partition
    for c in range(NCORES):
        for b in range(NBANKS):
            sel = np.flatnonzero(dst_valid[c] & (dst_all[c] // bank == b))
            # dedup: gather each distinct row once, point dup slots at it
            uniq, inv = np.unique(dst_all[c, sel], return_inverse=True)
            g = brow0[b] + np.arange(len(uniq))    # stage-A request index
            dstA_loc[c, g] = uniq - b * bank
            # request g lands at scratch flat row (g%128)*BC + g//128
            gslot = g[inv]
            dstB_row[c, sel] = (gslot % P) * BC + gslot // P

    def to_idx_tile(flat):
        n = flat.shape[1]
        blk = flat.reshape(NCORES, n // 16, 16).transpose(0, 2, 1)
        return np.tile(blk, (1, 8, 1)).astype(np.int16)

    src_idx = to_idx_tile(src_loc)
    dstA_idx = to_idx_tile(dstA_loc)
    dstB_idx = to_idx_tile(dstB_row)

    def chunk_plan(ranges, head=()):
        """head: sizes for the leading chunks of the first range (smaller
        chunks let the consumer start sooner)."""
        plan = []
        first = True
        for tag, a, b in ranges:
            g0 = a
            if first:
                for h in head:
                    if g0 + h > b:
                        break
                    plan.append((tag, g0, h))
                    g0 += h
                first = False
            while g0 < b:
                n = min(CHUNK, b - g0)
                plan.append((tag, g0, n))
                g0 += n
        return tuple(plan)

    # small leading src chunks: the PE pipeline (transpose->matmul) can
    # start as soon as the first 256 rows land instead of waiting ~9us
    src_plan = chunk_plan(
        [(s, seg_tile0[s] * P, seg_tile0[s + 1] * P) for s in range(NBANKS)],
        head=(256, 256, 512))
    dstA_plan = chunk_plan(
        [(b, brow0[b], brow0[b + 1]) for b in range(NBANKS)])
    # finer stage-B chunks: the per-supertile DVE stage consumes dst in
    # 1024-slot groups, so 512-row chunks smooth the tail
    dstB_plan = tuple(
        (0, g0, min(512, nslot - g0)) for g0 in range(0, nslot, 512))

    return (src_idx, dstA_idx, dstB_idx, relc_all, slot_edge_full, K, CT,
            src_plan, dstA_plan, dstB_plan, E)


def _build(K, CT, num_nodes, src_plan, dstA_plan, dstB_plan):
    nc = bacc.Bacc("TRN2", target_bir_lowering=False, debug=False,
                   num_devices=NCORES, num_swdge_queues=NQ)
    f32, i16 = mybir.dt.float32, mybir.dt.int16
    bank = -(-num_nodes // NBANKS)
    nslot = K * P
    BC = CT // P
    HG = TILE_GROUP // 2  # tiles per transpose/copy batch (one PSUM bank)

    node = nc.dram_tensor("node_emb", [num_nodes, DIM], f32,
                          kind="ExternalInput")
    wt_d = nc.dram_tensor("w_tile", [DIM, K * DIM], f32,
                          kind="ExternalInput")
    sidx_d = nc.dram_tensor("src_idx", [P, nslot // 16], i16,
                            kind="ExternalInput")
    daidx_d = nc.dram_tensor("dstA_idx", [P, CT // 16], i16,
                             kind="ExternalInput")
    dbidx_d = nc.dram_tensor("dstB_idx", [P, nslot // 16], i16,
                             kind="ExternalInput")
    out_d = nc.dram_tensor("scores", [P, K], f32, kind="ExternalOutput")

    with TileContext(nc) as tc:
        with (
            tc.tile_pool(name="persist", bufs=1) as persist,
            tc.tile_pool(name="tsb", bufs=6) as tsb_pool,
            tc.tile_pool(name="pbig", bufs=4) as pbig_pool,
            tc.tile_pool(name="dram", bufs=1, space="DRAM") as dram_pool,
            tc.tile_pool(name="tpsum", bufs=2, space="PSUM") as tpsum_pool,
            tc.tile_pool(name="upsum", bufs=6, space="PSUM") as upsum_pool,
        ):
            sidx = persist.tile([P, nslot // 16], i16, tag="sidx")
            daidx = persist.tile([P, CT // 16], i16, tag="daidx")
            dbidx = persist.tile([P, nslot // 16], i16, tag="dbidx")
            ident = persist.tile([P, P], f32, tag="ident")
            src_g = persist.tile([P, K * DIM], f32, tag="src_g")
            dstA = persist.tile([P, BC * DIM], f32, tag="dstA")
            dst_g = persist.tile([P, K * DIM], f32, tag="dst_g")
            w_g = persist.tile([DIM, K * DIM], f32, tag="w_g")
            scores = persist.tile([P, K], f32, tag="scores")
            scratch = dram_pool.tile([P, BC * DIM], f32, tag="scratch")

            nc.sync.dma_start(out=sidx[:], in_=sidx_d[:])
            nc.sync.dma_start(out=daidx[:], in_=daidx_d[:])
            nc.sync.dma_start(out=dbidx[:], in_=dbidx_d[:])
            nc.sync.dma_start(out=w_g[:], in_=wt_d[:])
            make_identity(nc, ident[:])

    # Issue order: small src head chunks first (unblock the PE pipeline),
            # then ALL dst-stage-A chunks (so the scratch hop + stage B start
            # early and overlap the src back-half), then the remaining src.
            # queue_num must follow the global SWDGE round-robin (Tile locks
            # DMA sem lanes to queues by instruction order).
            qn = 0
            src_items = [("s",) + it for it in src_plan]
            dstA_items = [("a",) + it for it in dstA_plan]
            nhead = sum(1 for _, _, n in src_plan if n < CHUNK and n <= 512)
            RUNWAY = 5  # src chunks pre-dispatched before the stage-B stall
            pre = (src_items[:nhead] + dstA_items
                   + src_items[nhead:nhead + RUNWAY])
            rest_src = src_items[nhead + RUNWAY:]

            def issue_gather(kind, bnk, g0, n):
                nonlocal_qn[0] += 1
                hi = min(num_nodes, (bnk + 1) * bank)
                g_tile, idx_tile = ((src_g, sidx) if kind == "s"
                                    else (dstA, daidx))
                nc.gpsimd.dma_gather(
                    g_tile[:, (g0 // P) * DIM:((g0 + n) // P) * DIM]
                    .rearrange("p (t d) -> p t d", d=DIM),
                    node[bnk * bank:hi, :],
                    idx_tile[:, g0 // 16:(g0 + n) // 16],
                    n, n, DIM,
                    queue_num=(nonlocal_qn[0] - 1) % NQ,
                )

            nonlocal_qn = [qn]
            for it in pre:
                issue_gather(*it)
            # stage A -> DRAM scratch (sequential, HWDGE path). Written in
            # per-chunk pieces so each piece streams out as soon as its
            # dstA gather lands, instead of one big write gated on ALL of
            # stage A — pulls the stage-B start earlier by most of the
            # write time. Every dstA chunk covers whole 128-row blocks
            # (brow0/bcaps are 128-multiples), so the SBUF slice is clean.
            for (_, g0, n) in dstA_plan:
                nc.sync.dma_start(
                    out=scratch[:, (g0 // P) * DIM:((g0 + n) // P) * DIM],
                    in_=dstA[:, (g0 // P) * DIM:((g0 + n) // P) * DIM],
                )
            # stage B interleaved with the remaining src chunks: the first
            # stage-B gather stalls the Pool sequencer on the write sem, but
            # PE has RUNWAY src chunks in flight; afterwards stage-B lands
            # incrementally so the DVE stage isn't starved at the tail
            scratch_rows = scratch[:].rearrange("a (b c) -> (a b) c", c=DIM)
            for ib, (_, g0, n) in enumerate(dstB_plan):
                nc.gpsimd.dma_gather(
                    dst_g[:, (g0 // P) * DIM:((g0 + n) // P) * DIM]
                    .rearrange("p (t d) -> p t d", d=DIM),
                    scratch_rows,
                    dbidx[:, g0 // 16:(g0 + n) // 16],
                    n, n, DIM,
                    queue_num=nonlocal_qn[0] % NQ,
                )
                nonlocal_qn[0] += 1
                if ib % 2 == 1 and rest_src:
                    issue_gather(*rest_src.pop(0))
            for it in rest_src:
                issue_gather(*it)

            for st in range(K // TILE_GROUP):
                t0 = st * TILE_GROUP
                # 4 pair-transposes ([128,128] -> [dA|dB, e]) into one PSUM
                # bank, then two strided ACT copies deinterleave the halves
                # into a base-partition-0 srcT buffer [64, 8*128].
                tp = tpsum_pool.tile([P, HG * P], f32, tag="tp")
                for q in range(HG):
                    c0 = (t0 + 2 * q) * DIM
                    nc.tensor.transpose(
                        out=tp[:, q * P:(q + 1) * P],
                        in_=src_g[:, c0:c0 + 2 * DIM],
                        identity=ident[:],
                    )
                tsb = tsb_pool.tile([DIM, TILE_GROUP * P], f32, tag="tsb")
                tsb_v = tsb[:].rearrange("p (t a c) -> p a t c", a=2, c=P)
                tp_v = tp[:].rearrange("p (q c) -> p q c", c=P)
                nc.scalar.copy(out=tsb_v[:, 0], in_=tp_v[0:DIM])
                nc.scalar.copy(out=tsb_v[:, 1], in_=tp_v[DIM:P])

                u = upsum_pool.tile([P, TILE_GROUP * DIM], f32, tag="u")
                for h in range(TILE_GROUP):
                    j = t0 + h
                    nc.tensor.matmul(
                        out=u[:, h * DIM:(h + 1) * DIM],
                        lhsT=tsb[:, h * P:(h + 1) * P],
                        rhs=w_g[:, j * DIM:(j + 1) * DIM],
                        start=True,
                        stop=True,
                    )
                pbig = pbig_pool.tile([P, TILE_GROUP * DIM], f32, tag="pbig")
                nc.vector.tensor_mul(
                    out=pbig[:],
                    in0=u[:],
                    in1=dst_g[:, t0 * DIM:(t0 + TILE_GROUP) * DIM],
                )
                nc.vector.reduce_sum(
                    out=scores[:, t0:t0 + TILE_GROUP],
                    in_=pbig[:].rearrange("p (t k) -> p t k", k=DIM),
                    axis=mybir.AxisListType.X,
                )

            nc.sync.dma_start(out=out_d[:], in_=scores[:])

    nc.compile()
    return nc


def kernel(triplets, node_emb, W):
    global LAST_RESULT
    node = np.ascontiguousarray(np.asarray(node_emb, dtype=np.float32))
    Wf = np.ascontiguousarray(np.asarray(W, dtype=np.float32))
    num_nodes = node.shape[0]

    (src_idx, dstA_idx, dstB_idx, relc_all, slot_edge, K, CT,
     src_plan, dstA_plan, dstB_plan, E) = _prepare(triplets, num_nodes)

    cache_key = (K, CT, num_nodes, src_plan, dstA_plan, dstB_plan)
    if cache_key not in _BUILD_CACHE:
        _BUILD_CACHE[cache_key] = _build(K, CT, num_nodes, src_plan,
                                         dstA_plan, dstB_plan)
    nc = _BUILD_CACHE[cache_key]

    in_maps = []
    for c in range(NCORES):
        # per-tile W: [K, 64, 64] -> [64, K*64] with w[d, j*64+k] = W[rel_j,d,k]
        wt = np.ascontiguousarray(
            Wf[relc_all[c]].transpose(1, 0, 2).reshape(DIM, K * DIM))
        in_maps.append({
            "node_emb": node,
            "w_tile": wt,
            "src_idx": np.ascontiguousarray(src_idx[c]),
            "dstA_idx": np.ascontiguousarray(dstA_idx[c]),
            "dstB_idx": np.ascontiguousarray(dstB_idx[c]),
        })

    res = run_bass_kernel_spmd(nc, in_maps, list(range(NCORES)), trace=TRACE)
    LAST_RESULT = res

    out = np.zeros(E, np.float32)
    for c in range(NCORES):
        sc = np.asarray(res.results[c]["scores"])  # [P, K]
        flat = sc.T.ravel()                        # index j*P+p = slot s
        se = slot_edge[c]
        valid = se >= 0
        out[se[valid]] = flat[valid]
    return out



# revision 4
# speedup vs baseline: 1.3068x; 1.3068x over previous
"""DistMult edge scoring on Trainium2 (8 NeuronCores).

score_e = src_emb[e]^T @ W[rel_e] @ dst_emb[e]   for E=100k edges.

Strategy
--------
Host (index-space preprocessing only — no embedding data is gathered on
host):
  - Sort edges by relation, shard the sorted list contiguously across the
    8 cores (data-parallel over edges).
  - Per core, bucket edges into 16 segments by (src_bank, dst_bank) where
    a bank is a num_nodes/4-row range of the node table (node ids must fit
    the int16 index of the HW gather instruction). Within a segment, edges
    stay relation-sorted and each relation run is padded to a multiple of
    128 so every 128-edge tile is single-relation and single-bank on both
    endpoints. Segment capacities are maxed across cores so all cores run
    one SPMD program.
  - The small relation matrices (64x64x64 = 1MB) are expanded per tile on
    host and streamed over the HWDGE path, keeping the SWDGE gather
    queues free for the 25.6MB node table.

Device (per core, SPMD):
  - dma_gather (int16, 4 SWDGE queues, <=1024 rows/instruction) of
    src/dst embedding rows, per bank segment.
  - per tile: PE transpose src -> srcT [d, e] (base partition 0);
    PE matmul U[e,k] = sum_d srcT[d,e] * W[r,d,k];
    per 8 tiles: DVE mul by dst and reduce over k -> scores.
Host: drop pad slots, unsort scores to the original edge order.
"""

import numpy as np

import concourse.bacc as bacc
import concourse.mybir as mybir
from concourse.bass_utils import run_bass_kernel_spmd
from concourse.masks import make_identity
from concourse.tile import TileContext

NCORES = 8
P = 128          # SBUF partitions / edges per tile
DIM = 64         # embedding dim
NUM_RELS = 64
NBANKS = 4
TILE_GROUP = 8   # tiles per compute super-tile (one PSUM bank: 8*64 f32)
CHUNK = 1024     # max indices per dma_gather instruction (SWDGE ring limit)
NQ = 4           # SWDGE queues

TRACE = False
LAST_RESULT = None

_BUILD_CACHE = {}


def _prepare(triplets, num_nodes):
    """Index-space prep. Returns per-core int16 gather streams + unsort map.

    Slot s (= tile j * 128 + p) of core c holds the edge at padded position
    s; tiles are relation- and bank-pure by construction.
    """
    t = np.asarray(triplets)
    E = t.shape[0]
    src = t[:, 0].astype(np.int64)
    rel = t[:, 1].astype(np.int64)
    dst = t[:, 2].astype(np.int64)
    bank = -(-num_nodes // NBANKS)  # equal banks, < 32768 for int16
    assert bank <= 32767

    order = np.argsort(rel, kind="stable")
    bounds = [round(c * E / NCORES) for c in range(NCORES + 1)]

    # segments by src bank only; dst goes through a two-stage gather
    core_data = []
    for c in range(NCORES):
        eidx = order[bounds[c]:bounds[c + 1]]
        seg = src[eidx] // bank
        segs = []
        for s in range(NBANKS):
            sel = eidx[seg == s]           # still rel-sorted (stable mask)
            # sort by (rel, src) within the segment: keeps rel runs
            # contiguous and makes gather rows ascending (DRAM locality)
            sel = sel[np.lexsort((src[sel], rel[sel]))]
            r = rel[sel]
            n = len(sel)
            if n:
                change = np.flatnonzero(np.diff(r)) + 1
                starts = np.concatenate([[0], change])
                ends = np.concatenate([change, [n]])
                lens = ends - starts
                padlens = ((lens + P - 1) // P) * P
                offs = np.concatenate([[0], np.cumsum(padlens)])
                total = int(offs[-1])
                se = np.full(total, -1, np.int64)
                pos = (np.arange(n) - np.repeat(starts, lens)
                       + np.repeat(offs[:-1], lens))
                se[pos] = sel
                sr = np.repeat(r[starts], padlens)
            else:
                se = np.zeros(0, np.int64)
                sr = np.zeros(0, np.int64)
            segs.append((se, sr))
        core_data.append(segs)

    caps = []
    for s in range(NBANKS):
        cap = max(len(core_data[c][s][0]) for c in range(NCORES)) // P
        caps.append(int(cap))
    K = sum(caps)
    pad_tiles = (-K) % TILE_GROUP  # compute loop works in groups of 8 tiles
    caps[-1] += pad_tiles
    K += pad_tiles
    seg_tile0 = np.concatenate([[0], np.cumsum(caps)]).astype(int)

    nslot = K * P
    src_loc = np.zeros((NCORES, nslot), np.int64)   # bank-local src idx
    dst_all = np.zeros((NCORES, nslot), np.int64)   # global dst ids
    dst_valid = np.zeros((NCORES, nslot), bool)
    relc_all = np.zeros((NCORES, K), np.int64)      # relation per tile
    slot_edge_full = np.full((NCORES, nslot), -1, np.int64)

    for c in range(NCORES):
        for s in range(NBANKS):
            se, sr = core_data[c][s]
            a = seg_tile0[s] * P
            m = len(se)
            slot_edge_full[c, a:a + m] = se
            valid = se >= 0
            sl = np.zeros(m, np.int64)
            sl[valid] = src[se[valid]] - s * bank
            src_loc[c, a:a + m] = sl
            dl = np.zeros(m, np.int64)
            dl[valid] = dst[se[valid]]
            dst_all[c, a:a + m] = dl
            dst_valid[c, a:a + m] = valid
            nt = m // P
            if nt:
                relc_all[c, seg_tile0[s]:seg_tile0[s] + nt] = \
                    sr.reshape(nt, P)[:, 0]

    # dst stage A: per dst bank, the slots needing that bank, compacted.
    # Per-bank capacity maxed over cores; total rows CT (scratch size).
    bank_cnt = np.zeros((NCORES, NBANKS), np.int64)
    for c in range(NCORES):
        for b in range(NBANKS):
            sel = dst_valid[c] & (dst_all[c] // bank == b)
            bank_cnt[c, b] = len(np.unique(dst_all[c, sel]))
    bcaps = [int(-(-bank_cnt[:, b].max() // P) * P) for b in range(NBANKS)]
    brow0 = np.concatenate([[0], np.cumsum(bcaps)]).astype(int)
    CT = int(brow0[-1])  # scratch rows; must fit int16 for stage B
    assert CT <= 32767 and CT % P == 0

    dstA_loc = np.zeros((NCORES, CT), np.int64)   # stage-A bank-local ids
    dstB_row = np.zeros((NCORES, nslot), np.int64)  # stage-B scratch rows
    BC = CT // P  # scratch free blocks per partition
    for c in range(NCORES):
        for b in range(NBANKS):
            sel = np.flatnonzero(dst_valid[c] & (dst_all[c] // bank == b))
            # dedup: gather each distinct row once, point dup slots at it
            uniq, inv = np.unique(dst_all[c, sel], return_inverse=True)
            g = brow0[b] + np.arange(len(uniq))    # stage-A request index
            dstA_loc[c, g] = uniq - b * bank
            # request g lands at scratch flat row (g%128)*BC + g//128
            gslot = g[inv]
            dstB_row[c, sel] = (gslot % P) * BC + gslot // P

    def to_idx_tile(flat):
        n = flat.shape[1]
        blk = flat.reshape(NCORES, n // 16, 16).transpose(0, 2, 1)
        return np.tile(blk, (1, 8, 1)).astype(np.int16)

    src_idx = to_idx_tile(src_loc)
    dstA_idx = to_idx_tile(dstA_loc)
    dstB_idx = to_idx_tile(dstB_row)

    def chunk_plan(ranges, head=()):
        """head: sizes for the leading chunks of the first range (smaller
        chunks let the consumer start sooner)."""
        plan = []
        first = True
        for tag, a, b in ranges:
            g0 = a
            if first:
                for h in head:
                    if g0 + h > b:
                        break
                    plan.append((tag, g0, h))
                    g0 += h
                first = False
            while g0 < b:
                n = min(CHUNK, b - g0)
                plan.append((tag, g0, n))
                g0 += n
        return tuple(plan)

    # small leading src chunks: the PE pipeline (transpose->matmul) can
    # start as soon as the first 256 rows land instead of waiting ~9us
    src_plan = chunk_plan(
        [(s, seg_tile0[s] * P, seg_tile0[s + 1] * P) for s in range(NBANKS)],
        head=(256, 256, 512))
    dstA_plan = chunk_plan(
        [(b, brow0[b], brow0[b + 1]) for b in range(NBANKS)])
    # finer stage-B chunks: the per-supertile DVE stage consumes dst in
    # 1024-slot groups, so 512-row chunks smooth the tail
    dstB_plan = tuple(
        (0, g0, min(512, nslot - g0)) for g0 in range(0, nslot, 512))

    return (src_idx, dstA_idx, dstB_idx, relc_all, slot_edge_full, K, CT,
            src_plan, dstA_plan, dstB_plan, E)


def _build(K, CT, num_nodes, src_plan, dstA_plan, dstB_plan):
    nc = bacc.Bacc("TRN2", target_bir_lowering=False, debug=False,
                   num_devices=NCORES, num_swdge_queues=NQ)
    f32, i16 = mybir.dt.float32, mybir.dt.int16
    bank = -(-num_nodes // NBANKS)
    nslot = K * P
    BC = CT // P
    HG = TILE_GROUP // 2  # tiles per transpose/copy batch (one PSUM bank)

    node = nc.dram_tensor("node_emb", [num_nodes, DIM], f32,
                          kind="ExternalInput")
    wt_d = nc.dram_tensor("w_tile", [DIM, K * DIM], f32,
                          kind="ExternalInput")
    sidx_d = nc.dram_tensor("src_idx", [P, nslot // 16], i16,
                            kind="ExternalInput")
    daidx_d = nc.dram_tensor("dstA_idx", [P, CT // 16], i16,
                             kind="ExternalInput")
    dbidx_d = nc.dram_tensor("dstB_idx", [P, nslot // 16], i16,
                             kind="ExternalInput")
    out_d = nc.dram_tensor("scores", [P, K], f32, kind="ExternalOutput")

    with TileContext(nc) as tc:
        with (
            tc.tile_pool(name="persist", bufs=1) as persist,
            tc.tile_pool(name="tsb", bufs=6) as tsb_pool,
            tc.tile_pool(name="pbig", bufs=4) as pbig_pool,
            tc.tile_pool(name="dram", bufs=1, space="DRAM") as dram_pool,
            tc.tile_pool(name="tpsum", bufs=2, space="PSUM") as tpsum_pool,
            tc.tile_pool(name="upsum", bufs=6, space="PSUM") as upsum_pool,
        ):
            sidx = persist.tile([P, nslot // 16], i16, tag="sidx")
            daidx = persist.tile([P, CT // 16], i16, tag="daidx")
            dbidx = persist.tile([P, nslot // 16], i16, tag="dbidx")
            ident = persist.tile([P, P], f32, tag="ident")
            src_g = persist.tile([P, K * DIM], f32, tag="src_g")
            dstA = persist.tile([P, BC * DIM], f32, tag="dstA")
            dst_g = persist.tile([P, K * DIM], f32, tag="dst_g")
            w_g = persist.tile([DIM, K * DIM], f32, tag="w_g")
            scores = persist.tile([P, K], f32, tag="scores")
            scratch = dram_pool.tile([P, BC * DIM], f32, tag="scratch")

            nc.sync.dma_start(out=sidx[:], in_=sidx_d[:])
            nc.sync.dma_start(out=daidx[:], in_=daidx_d[:])
            nc.sync.dma_start(out=dbidx[:], in_=dbidx_d[:])
            nc.sync.dma_start(out=w_g[:], in_=wt_d[:])
            make_identity(nc, ident[:])

    # Issue order: small src head chunks first (unblock the PE pipeline),
            # then ALL dst-stage-A chunks (so the scratch hop + stage B start
            # early and overlap the src back-half), then the remaining src.
            # queue_num must follow the global SWDGE round-robin (Tile locks
            # DMA sem lanes to queues by instruction order).
            qn = 0
            src_items = [("s",) + it for it in src_plan]
            dstA_items = [("a",) + it for it in dstA_plan]
            nhead = sum(1 for _, _, n in src_plan if n < CHUNK and n <= 512)
            RUNWAY = 5  # src chunks pre-dispatched before the stage-B stall
            pre = (src_items[:nhead] + dstA_items
                   + src_items[nhead:nhead + RUNWAY])
            rest_src = src_items[nhead + RUNWAY:]

            def issue_gather(kind, bnk, g0, n):
                nonlocal_qn[0] += 1
                hi = min(num_nodes, (bnk + 1) * bank)
                g_tile, idx_tile = ((src_g, sidx) if kind == "s"
                                    else (dstA, daidx))
                nc.gpsimd.dma_gather(
                    g_tile[:, (g0 // P) * DIM:((g0 + n) // P) * DIM]
                    .rearrange("p (t d) -> p t d", d=DIM),
                    node[bnk * bank:hi, :],
                    idx_tile[:, g0 // 16:(g0 + n) // 16],
                    n, n, DIM,
                    queue_num=(nonlocal_qn[0] - 1) % NQ,
                )

            nonlocal_qn = [qn]
            for it in pre:
                issue_gather(*it)
            # stage A -> DRAM scratch (sequential, HWDGE path). Written in
            # per-chunk pieces so each piece streams out as soon as its
            # dstA gather lands, instead of one big write gated on ALL of
            # stage A — pulls the stage-B start earlier by most of the
            # write time. Every dstA chunk covers whole 128-row blocks
            # (brow0/bcaps are 128-multiples), so the SBUF slice is clean.
            for (_, g0, n) in dstA_plan:
                nc.sync.dma_start(
                    out=scratch[:, (g0 // P) * DIM:((g0 + n) // P) * DIM],
                    in_=dstA[:, (g0 // P) * DIM:((g0 + n) // P) * DIM],
                )
            # stage B interleaved with the remaining src chunks: the first
            # stage-B gather stalls the Pool sequencer on the write sem, but
            # PE has RUNWAY src chunks in flight; afterwards stage-B lands
            # incrementally so the DVE stage isn't starved at the tail
            scratch_rows = scratch[:].rearrange("a (b c) -> (a b) c", c=DIM)
            for ib, (_, g0, n) in enumerate(dstB_plan):
                nc.gpsimd.dma_gather(
                    dst_g[:, (g0 // P) * DIM:((g0 + n) // P) * DIM]
                    .rearrange("p (t d) -> p t d", d=DIM),
                    scratch_rows,
                    dbidx[:, g0 // 16:(g0 + n) // 16],
                    n, n, DIM,
                    queue_num=nonlocal_qn[0] % NQ,
                )
                nonlocal_qn[0] += 1
                if ib % 2 == 1 and rest_src:
                    issue_gather(*rest_src.pop(0))
            for it in rest_src:
                issue_gather(*it)

            for st in range(K // TILE_GROUP):
                t0 = st * TILE_GROUP
                # 4 pair-transposes ([128,128] -> [dA|dB, e]) into one PSUM
                # bank, then two strided ACT copies deinterleave the halves
                # into a base-partition-0 srcT buffer [64, 8*128].
                tp = tpsum_pool.tile([P, HG * P], f32, tag="tp")
                for q in range(HG):
                    c0 = (t0 + 2 * q) * DIM
                    nc.tensor.transpose(
                        out=tp[:, q * P:(q + 1) * P],
                        in_=src_g[:, c0:c0 + 2 * DIM],
                        identity=ident[:],
                    )
                tsb = tsb_pool.tile([DIM, TILE_GROUP * P], f32, tag="tsb")
                tsb_v = tsb[:].rearrange("p (t a c) -> p a t c", a=2, c=P)
                tp_v = tp[:].rearrange("p (q c) -> p q c", c=P)
                nc.scalar.copy(out=tsb_v[:, 0], in_=tp_v[0:DIM])
                nc.scalar.copy(out=tsb_v[:, 1], in_=tp_v[DIM:P])

                u = upsum_pool.tile([P, TILE_GROUP * DIM], f32, tag="u")
                for h in range(TILE_GROUP):
                    j = t0 + h
                    nc.tensor.matmul(
                        out=u[:, h * DIM:(h + 1) * DIM],
                        lhsT=tsb[:, h * P:(h + 1) * P],
                        rhs=w_g[:, j * DIM:(j + 1) * DIM],
                        start=True,
                        stop=True,
                    )
                pbig = pbig_pool.tile([P, TILE_GROUP * DIM], f32, tag="pbig")
                nc.vector.tensor_mul(
                    out=pbig[:],
                    in0=u[:],
                    in1=dst_g[:, t0 * DIM:(t0 + TILE_GROUP) * DIM],
                )
                nc.vector.reduce_sum(
                    out=scores[:, t0:t0 + TILE_GROUP],
                    in_=pbig[:].rearrange("p (t k) -> p t k", k=DIM),
                    axis=mybir.AxisListType.X,
                )

            nc.sync.dma_start(out=out_d[:], in_=scores[:])

    nc.compile()
    return nc


def kernel(triplets, node_emb, W):
    global LAST_RESULT
    node = np.ascontiguousarray(np.asarray(node_emb, dtype=np.float32))
    Wf = np.ascontiguousarray(np.asarray(W, dtype=np.float32))
    num_nodes = node.shape[0]

    (src_idx, dstA_idx, dstB_idx, relc_all, slot_edge, K, CT,
     src_plan, dstA_plan, dstB_plan, E) = _prepare(triplets, num_nodes)

    cache_key = (K, CT, num_nodes, src_plan, dstA_plan, dstB_plan)
    if cache_key not in _BUILD_CACHE:
        _BUILD_CACHE[cache_key] = _build(K, CT, num_nodes, src_plan,
                                         dstA_plan, dstB_plan)
    nc = _BUILD_CACHE[cache_key]

    in_maps = []
    for c in range(NCORES):
        # per-tile W: [K, 64, 64] -> [64, K*64] with w[d, j*64+k] = W[rel_j,d,k]
        wt = np.ascontiguousarray(
            Wf[relc_all[c]].transpose(1, 0, 2).reshape(DIM, K * DIM))
        in_maps.append({
            "node_emb": node,
            "w_tile": wt,
            "src_idx": np.ascontiguousarray(src_idx[c]),
            "dstA_idx": np.ascontiguousarray(dstA_idx[c]),
            "dstB_idx": np.ascontiguousarray(dstB_idx[c]),
        })

    res = run_bass_kernel_spmd(nc, in_maps, list(range(NCORES)), trace=TRACE)
    LAST_RESULT = res

    out = np.zeros(E, np.float32)
    for c in range(NCORES):
        sc = np.asarray(res.results[c]["scores"])  # [P, K]
        flat = sc.T.ravel()                        # index j*P+p = slot s
        se = slot_edge[c]
        valid = se >= 0
        out[se[valid]] = flat[valid]
    return out



# revision 5
# speedup vs baseline: 1.4125x; 1.0809x over previous
"""DistMult edge scoring on Trainium2 (8 NeuronCores) — v6.

score_e = src_emb[e]^T @ W[rel_e] @ dst_emb[e]   for E=100k edges.

v6: transposed-space compute => near-zero slot padding.
-------------------------------------------------------
v2 was SDMA-bound with 28% of gather rows being padding (tiles had to
be relation-pure for the per-tile W matmul). v6 computes in
"transposed" space where the relation only selects matmul COLUMN
ranges, which are free-form:

  srcT[d, e]  (PE transpose of gathered src rows)
  v[k, e]    = sum_d W[r_e][d, k] * srcT[d, e]   (matmul per rel-run,
               lhsT = W[r] [d,k], rhs = srcT column slice — arbitrary
               column ranges, so rel runs need NO padding)
  z[k, e]    = v[k, e] * dstT[k, e]              (DVE)
  score[e]   = sum_k z[k, e]                     (PE ones-matmul per
               128-block: lhsT = z block, rhs = ones[64,1])

Host (index-space preprocessing only):
  - Node table in 4 banks of B rows (int16 gather index limit). Edge ->
    core by (src_bank s, dst_half j): core 2s+j owns dst banks
    {2j, 2j+1}. Per-core slot order: (dst_half, rel, src_id); only the
    dst_half boundary and the total are 128-aligned. Fill ~97%.
  - Uploads per core: src_bank [B,64], dst_banks [2B,64] (bank slices,
    so SPMD gather ranges are program-constant), idx tiles, W
    (raw [64, 64*64], relation selects columns), rel-run table baked
    into the program.

Device: dma_gather src+dst rows (~2*12900 rows/core, 4 SWDGE queues),
pipeline above per 512-slot supertile.
Host: drop pad slots, unsort scores to the original edge order.
"""

import numpy as np

import concourse.bacc as bacc
import concourse.mybir as mybir
from concourse.bass_utils import run_bass_kernel_spmd
from concourse.masks import make_identity
from concourse.tile import TileContext

NCORES = 8
P = 128
DIM = 64
NBANKS = 4
ST = 512         # slots per supertile (one PSUM bank of f32 [64, 512])
CHUNK = 512      # half the SWDGE ring per queue: 2 gathers in flight/queue
NQ = 4

TRACE = False
LAST_RESULT = None

_BUILD_CACHE = {}


def _prepare(triplets, num_nodes):
    t = np.asarray(triplets)
    E = t.shape[0]
    src = t[:, 0].astype(np.int64)
    rel = t[:, 1].astype(np.int64)
    dst = t[:, 2].astype(np.int64)
    B = -(-num_nodes // NBANKS)
    assert B <= 32767
    sb = src // B
    db = dst // B
    core_of_edge = sb * 2 + db // 2

    # Slot order: (dst_half h, rel r, src). Each (h, r) run is padded to
    # the cross-core max length so rel-run boundaries are IDENTICAL on
    # all cores (SPMD: matmul column ranges are program constants and
    # the relation of a segment is core-independent). Only the h
    # boundary and the total are 128/ST-aligned. Fill ~87%.
    NR = 64
    sel_hr = [[[None] * NR for _ in range(2)] for _ in range(NCORES)]
    cnt = np.zeros((NCORES, 2, NR), np.int64)
    for c in range(NCORES):
        j = c % 2
        for h in range(2):
            selh = np.flatnonzero((core_of_edge == c) & (db == 2 * j + h))
            rh = rel[selh]
            order = np.lexsort((src[selh], rh))
            selh = selh[order]
            rh = rh[order]
            starts = np.searchsorted(rh, np.arange(NR))
            ends = np.searchsorted(rh, np.arange(NR) + 1)
            for r in range(NR):
                sel_hr[c][h][r] = selh[starts[r]:ends[r]]
                cnt[c, h, r] = ends[r] - starts[r]

    caps = cnt.max(axis=0)              # [2, NR] cross-core run capacity
    # align the h=0 half and the total to 128 / ST by padding last runs
    half0 = int(caps[0].sum())
    caps[0, NR - 1] += (-half0) % P
    cap0 = int(caps[0].sum())
    total = cap0 + int(caps[1].sum())
    caps[1, NR - 1] += (-total) % ST
    nslot = cap0 + int(caps[1].sum())
    K = nslot // P

    run_off = np.zeros((2, NR), np.int64)
    off = 0
    segs = []   # (start, end, rel) — same for every core
    for h in range(2):
        for r in range(NR):
            run_off[h, r] = off
            if caps[h, r]:
                segs.append((int(off), int(off + caps[h, r]), r))
            off += caps[h, r]
    # split segments at ST boundaries (PSUM supertile granularity)
    segs2 = []
    for (a, b, r) in segs:
        while a // ST != (b - 1) // ST:
            cut = (a // ST + 1) * ST
            segs2.append((a, cut, r))
            a = cut
        segs2.append((a, b, r))
    segs = tuple(segs2)

    slot_edge = np.full((NCORES, nslot), -1, np.int64)
    src_loc = np.zeros((NCORES, nslot), np.int64)
    dst_loc = np.zeros((NCORES, nslot), np.int64)
    for c in range(NCORES):
        for h in range(2):
            for r in range(NR):
                se = sel_hr[c][h][r]
                m = len(se)
                if not m:
                    continue
                a = run_off[h, r]
                slot_edge[c, a:a + m] = se
                src_loc[c, a:a + m] = src[se] % B
                dst_loc[c, a:a + m] = dst[se] % B

    def to_idx_tile(flat):
        n = flat.shape[1]
        blk = flat.reshape(NCORES, n // 16, 16).transpose(0, 2, 1)
        return np.tile(blk, (1, 8, 1)).astype(np.int16)

    src_idx = to_idx_tile(src_loc)
    dst_idx = to_idx_tile(dst_loc)

    def chunks(a, b, head=(), tail=()):
        plan = []
        g0 = a
        for hd in head:
            if g0 + hd > b:
                break
            plan.append((g0, hd))
            g0 += hd
        tail_total = sum(tail)
        cut = b - tail_total if b - g0 > tail_total + CHUNK // 2 else b
        while g0 < cut:
            n = min(CHUNK, cut - g0)
            plan.append((g0, n))
            g0 += n
        for tl in tail:
            if g0 >= b:
                break
            n = min(tl, b - g0)
            plan.append((g0, n))
            g0 += n
        while g0 < b:
            n = min(CHUNK, b - g0)
            plan.append((g0, n))
            g0 += n
        return plan

    src_plan = tuple(chunks(0, nslot, head=(256, 256, 512),
                            tail=(512, 256, 256)))
    dst_plan = tuple([(0,) + it for it in chunks(0, cap0)]
                     + [(1,) + it for it in
                        chunks(cap0, nslot, tail=(512, 256, 256))])

    return (src_idx, dst_idx, slot_edge, K, cap0, segs,
            src_plan, dst_plan, E, B)


def _build(K, cap0, segs, B, src_plan, dst_plan):
    nc = bacc.Bacc("TRN2", target_bir_lowering=False, debug=False,
                   num_devices=NCORES, num_swdge_queues=NQ)
    f32, i16 = mybir.dt.float32, mybir.dt.int16
    nslot = K * P
    NST = nslot // ST

    sbank = nc.dram_tensor("src_bank", [B, DIM], f32, kind="ExternalInput")
    dbank = nc.dram_tensor("dst_banks", [2 * B, DIM], f32,
                           kind="ExternalInput")
    bf16 = mybir.dt.bfloat16
    # raw W (bf16), [64, 64*64]: segment rel selects a 64-column slice
    wall_d = nc.dram_tensor("w_all", [DIM, 64 * DIM], bf16,
                            kind="ExternalInput")
    sidx_d = nc.dram_tensor("src_idx", [P, nslot // 16], i16,
                            kind="ExternalInput")
    didx_d = nc.dram_tensor("dst_idx", [P, nslot // 16], i16,
                            kind="ExternalInput")
    out_d = nc.dram_tensor("scores", [1, K * P], f32,
                           kind="ExternalOutput")

    with TileContext(nc) as tc:
        with (
            tc.tile_pool(name="persist", bufs=1) as persist,
            tc.tile_pool(name="tsb", bufs=6) as tsb_pool,
            tc.tile_pool(name="zsb", bufs=6) as zsb_pool,
            tc.tile_pool(name="tpsum", bufs=3, space="PSUM") as tpsum_pool,
            tc.tile_pool(name="vpsum", bufs=3, space="PSUM") as vpsum_pool,
            tc.tile_pool(name="spsum", bufs=2, space="PSUM") as spsum_pool,
        ):
            sidx = persist.tile([P, nslot // 16], i16, tag="sidx")
            didx = persist.tile([P, nslot // 16], i16, tag="didx")
            ident = persist.tile([P, P], f32, tag="ident")
            ones = persist.tile([DIM, 1], bf16, tag="ones")
            ones_d = nc.dram_tensor("ones", [DIM, 1], bf16,
                                    kind="ExternalInput")
            src_g = persist.tile([P, K * DIM], f32, tag="src_g")
            dst_g = persist.tile([P, K * DIM], f32, tag="dst_g")
            w_g = persist.tile([DIM, 64 * DIM], bf16, tag="w_g")
            scores = persist.tile([1, K * P], f32, tag="scores")

            wu_d = nc.dram_tensor("wu_idx", [P, 1], i16,
                                  kind="ExternalInput")
            wuidx = persist.tile([P, 1], i16, tag="wuidx")
            wug = persist.tile([P, DIM], f32, tag="wug")
            nc.sync.dma_start(out=wuidx[:], in_=wu_d[:])
            # warmup: the first SWDGE instruction pays ~9us of Q7 ucode
            # warmup; run it on a 16-row dummy while the idx tiles load
            nc.gpsimd.dma_gather(
                wug[:].rearrange("p (t d) -> p t d", d=DIM),
                sbank[:], wuidx[:], 16, 16, DIM, queue_num=0)
            nc.sync.dma_start(out=sidx[:], in_=sidx_d[:])
            nc.sync.dma_start(out=didx[:], in_=didx_d[:])
            nc.sync.dma_start(out=w_g[:], in_=wall_d[:])
            nc.sync.dma_start(out=ones[:], in_=ones_d[:])
            make_identity(nc, ident[:])

            qn = [0]

            def gather(g_tile, idx_tile, table, g0, n):
                nc.gpsimd.dma_gather(
                    g_tile[:, (g0 // P) * DIM:((g0 + n + P - 1) // P) * DIM]
                    .rearrange("p (t d) -> p t d", d=DIM),
                    table,
                    idx_tile[:, g0 // 16:(g0 + n) // 16],
                    n, n, DIM,
                    queue_num=qn[0] % NQ,
                )
                qn[0] += 1

            src_items = list(src_plan)
            dst_items = list(dst_plan)
            nhead = sum(1 for _, n in src_plan if n < CHUNK)
            for g0, n in src_items[:nhead]:
                gather(src_g, sidx, sbank[:], g0, n)
            src_items = src_items[nhead:]
            while src_items or dst_items:
                if dst_items:
                    h, g0, n = dst_items.pop(0)
                    gather(dst_g, didx, dbank[h * B:(h + 1) * B, :], g0, n)
                if src_items:
                    g0, n = src_items.pop(0)
                    gather(src_g, sidx, sbank[:], g0, n)

            # segments grouped by supertile
            seg_by_st = [[] for _ in range(NST)]
            for (a, b, r) in segs:
                seg_by_st[a // ST].append((a, b, r))

            for st in range(NST):
                s0 = st * ST
                # transpose src and dst 128-blocks of this supertile into
                # PSUM, deinterleave into [64, ST] SBUF tiles
                tp = tpsum_pool.tile([P, 4 * P], f32, tag="tp")
                srcT = tsb_pool.tile([DIM, ST], bf16, tag="srcT")
                dstT = tsb_pool.tile([DIM, ST], f32, tag="dstT")
                for q in range(2):
                    c0 = (s0 // P + 2 * q) * DIM
                    nc.tensor.transpose(
                        out=tp[:, q * P:(q + 1) * P],
                        in_=src_g[:, c0:c0 + 2 * DIM],
                        identity=ident[:],
                    )
                    nc.tensor.transpose(
                        out=tp[:, (2 + q) * P:(3 + q) * P],
                        in_=dst_g[:, c0:c0 + 2 * DIM],
                        identity=ident[:],
                    )
                for half, tt, eng in ((0, srcT, nc.scalar),
                                      (1, dstT, nc.vector)):
                    tt_v = tt[:].rearrange("p (t a c) -> p a t c", a=2, c=P)
                    tp_v = tp[:, half * 2 * P:(half + 1) * 2 * P] \
                        .rearrange("p (q c) -> p q c", c=P)
                    if eng is nc.scalar:
                        eng.copy(out=tt_v[:, 0], in_=tp_v[0:DIM])
                        eng.copy(out=tt_v[:, 1], in_=tp_v[DIM:P])
                    else:
                        eng.tensor_copy(out=tt_v[:, 0], in_=tp_v[0:DIM])
                        eng.tensor_copy(out=tt_v[:, 1], in_=tp_v[DIM:P])

                v = vpsum_pool.tile([DIM, ST], f32, tag="v")
                for (a, b, r) in seg_by_st[st]:
                    nc.tensor.matmul(
                        out=v[:, a - s0:b - s0],
                        lhsT=w_g[:, r * DIM:(r + 1) * DIM],
                        rhs=srcT[:, a - s0:b - s0],
                        start=True,
                        stop=True,
                    )
                z = zsb_pool.tile([DIM, ST], bf16, tag="z")
                nc.vector.tensor_mul(out=z[:], in0=v[:], in1=dstT[:])
                sc = spsum_pool.tile([1, ST], f32, tag="sc")
                nc.tensor.matmul(
                    out=sc[:],
                    lhsT=ones[:],
                    rhs=z[:],
                    start=True,
                    stop=True,
                )
                nc.scalar.copy(
                    out=scores[:, st * ST:(st + 1) * ST],
                    in_=sc[:],
                )

            nc.sync.dma_start(out=out_d[:], in_=scores[:])

    nc.compile()
    return nc


def kernel(triplets, node_emb, W):
    global LAST_RESULT
    node = np.ascontiguousarray(np.asarray(node_emb, dtype=np.float32))
    Wf = np.ascontiguousarray(np.asarray(W, dtype=np.float32))
    num_nodes = node.shape[0]

    (src_idx, dst_idx, slot_edge, K, cap0, segs,
     src_plan, dst_plan, E, B) = _prepare(triplets, num_nodes)

    cache_key = (K, cap0, segs, B, src_plan, dst_plan)
    if cache_key not in _BUILD_CACHE:
        _BUILD_CACHE[cache_key] = _build(K, cap0, segs, B, src_plan,
                                         dst_plan)
    nc = _BUILD_CACHE[cache_key]

    banks = []
    for b in range(NBANKS):
        blk = node[b * B:(b + 1) * B]
        if blk.shape[0] < B:
            blk = np.vstack([blk, np.zeros((B - blk.shape[0], DIM),
                                           np.float32)])
        banks.append(np.ascontiguousarray(blk))

    import ml_dtypes
    w_all = np.ascontiguousarray(
        Wf.transpose(1, 0, 2).reshape(DIM, 64 * DIM)
        .astype(ml_dtypes.bfloat16))
    in_maps = []
    for c in range(NCORES):
        s, j = c // 2, c % 2
        in_maps.append({
            "src_bank": banks[s],
            "dst_banks": np.ascontiguousarray(
                np.vstack([banks[2 * j], banks[2 * j + 1]])),
            "w_all": w_all,
            "ones": np.ones((DIM, 1), ml_dtypes.bfloat16),
            "wu_idx": np.zeros((P, 1), np.int16),
            "src_idx": np.ascontiguousarray(src_idx[c]),
            "dst_idx": np.ascontiguousarray(dst_idx[c]),
        })

    res = run_bass_kernel_spmd(nc, in_maps, list(range(NCORES)), trace=TRACE)
    LAST_RESULT = res

    out = np.zeros(E, np.float32)
    for c in range(NCORES):
        flat = np.asarray(res.results[c]["scores"]).ravel()  # [nslot]
        se = slot_edge[c]
        valid = se >= 0
        out[se[valid]] = flat[valid]
    return out
